# revision 12
# baseline (speedup 1.0000x reference)
"""Block-diagonal MLP kernel for TRN2, 8 NeuronCores.

Computes out = x @ tanh(blocks * mask) where blocks is 4096x4096 with 16
diagonal 256x256 blocks. Off-diagonal entries of tanh(blocks*mask) are
tanh(0)=0, so only the 16 diagonal blocks matter:

    out[:, 256k:256(k+1)] = x[:, 256k:256(k+1)] @ tanh(B_k)

Sharding: block-parallel. Core c owns blocks 2c and 2c+1 (512 contiguous
k/n-columns) and streams all 8192 rows of x:

    outT_shard[n, m] = sum_k b[k, n] * xT_shard[k, m]      (n, k local)

Wire format: x ships as fp8e3 (e3m4, 1.34% rms quant err) and feeds the
PE DIRECTLY as the moving operand of a mixed-dtype matmul against bf16
weights (verified exact on HW vs numpy) -- no on-chip cast at all. The
output returns as int8 with a per-column scale s_o[n] =
4*||tanh(B)[:,n]||*std(x)/127 folded into the weights (wsb = tanh(B)/s_o
bf16), so PSUM holds the int8 output value directly and the DVE/ACT
evacuation cast (round-to-nearest-even + saturate) finishes the
quantization for free. End-to-end rel l2 err 1.66e-2 (gate 2e-2),
matching the numpy simulation of the scheme.

DMA economics on TRN2 (measured): a dma_start costs ~0.65us of serial
issue time on its engine (HWDGE descriptor gen runs inside the
instruction), a ring takes ~0.8-2.5us from doorbell to first byte, the
completion semaphore fires ~0.3-0.5us after the last byte, and the 16
SDMA engines round-robin over ALL queues with pending work at packet
granularity -- so anything else in flight stretches the transfer you
are waiting on. Hence the whole critical path rides ONE ring (SP
HWDGE) as a FIFO in exact consumption order, with the host laying out
both x and the weights so each DMA is one contiguous block whose
completion unblocks the next slice of matmuls:

    b1a = q0[0:2048]   (first psum-pair kc0 data)   256 KiB
    w1  = weight cols for blk0 (consumption-ordered) 64 KiB
    b1b = q1[0:2048]   (kc1)                         256 KiB
    w2  = remaining weight cols                     192 KiB
    b2a = q1[2048:4096] (zigzag: mh2=1 runs kc1 first), b2b, b3, b4

Group-1 x (needed after ~25us) goes SWDGE (gpsimd) as two 1 MiB
contiguous blobs, gated behind w2's arrival by a tiny gpsimd copy so
its packets don't steal wire from the critical head. Stores ride the
ACT ring (woken early by a dummy DMA), alternating with the SP ring
once it is idle; the final store splits across both rings. First real
matmul lands ~10us; warm-up matmuls from the end of the PE preamble
(~8.4us) start the HAM activity window early so the stream is at 2.4
GHz from ~11.8us and runs gapless to the end.

Matmuls run bf16(stationary) x fp8e3(moving) with fp32 PSUM
accumulation over k=256 (2 chained 128-row matmuls); psum pairs use a
zigzag kc order so consecutive pairs share the stationary weight (f32
a+b is exactly commutative). PSUM evacuations alternate DVE/ACT.
"""

import ml_dtypes
import numpy as np

import concourse.mybir as mybir
import concourse.tile as tile
from concourse import bacc
from concourse.bass_utils import run_bass_kernel_spmd

N_CORES = 8
N_ROWS = 8192            # rows of x / out
D = 4096                 # layer size
BLOCK = 256              # block size
BLOCKS_PER_CORE = 2      # 16 blocks / 8 cores
K_PER_CORE = BLOCKS_PER_CORE * BLOCK   # 512 k (and n) columns per core

O_CLIP = 4.0             # clip out column n at 4 sigma_n (per-column scale)

M_GROUP = 4096           # m columns per store tile / n-col sweep
N_GROUPS = N_ROWS // M_GROUP
MM_FREE = 512            # matmul moving free dim (one fp32 PSUM bank)
HALF = M_GROUP // 2      # m columns per packed half-blob (per q)

WARMUP_MMS = 14          # no-dep matmuls (N=256): start the HAM activity
WARMUP_FREE = 256        # window at PE-preamble end (~7.3us) and bridge
                         # GAPLESS to first data (~10.7us) so the real stream
                         # starts already at 2.4 GHz
DVE_EVAC_SLOTS = (0, 2, 4, 6)  # evacs alternate DVE/ACT evenly

_nc_cache = None


def _build_nc():
    f32 = mybir.dt.float32
    bf16 = mybir.dt.bfloat16
    f8e3 = mybir.dt.float8e3
    i8 = mybir.dt.int8

    nc = bacc.Bacc("TRN2")
    # consumption-ordered packed x (host-prepared, see _make_in_maps):
    # g0: xpk0[bp, h, p, slot*2048+j] = xT[q][p, h*2048+j] where the slot
    #     order is (q_even, q_odd) for h=0 and (q_odd, q_even) for h=1
    #     (zigzag kc order consumes the odd q first on h=1)
    # g1: xpk1[bp, p, h*4096 + slot*2048 + j] -- same packing, fused so each
    #     bp is ONE contiguous 1 MiB SWDGE blob
    xpk0 = nc.dram_tensor("xpk0", [2, 2, 128, M_GROUP], f8e3,
                          kind="ExternalInput")
    xpk1 = nc.dram_tensor("xpk1", [2, 128, 2 * M_GROUP], f8e3,
                          kind="ExternalInput")
    # host-prepped weights, consumption-ordered:
    # wsb[p, ((blk*2+ncol)*2+kc)*128 + n128] =
    #     tanh(B_blk)[kc*128+p, ncol*128+n128] / s_o[...]
    wsb = nc.dram_tensor("wsb", [128, 1024], bf16, kind="ExternalInput")
    outTt = nc.dram_tensor("outTt", [N_GROUPS, BLOCKS_PER_CORE, 2, 128, M_GROUP],
                           i8, kind="ExternalOutput")

    with tile.TileContext(nc) as tc:
        with (
            tc.tile_pool(name="wpool", bufs=1) as wpool,
            tc.tile_pool(name="xpool", bufs=4) as xpool,
            tc.tile_pool(name="opool", bufs=6) as opool,
            tc.tile_pool(name="pspool", bufs=4, space="PSUM") as pspool,
        ):
            # --- ACT ring wake-up + weights: the dummy absorbs the ring-start
            # latency; the weights then ride this ring in parallel with the
            # x stream on the SP ring (w1 = first-consumed columns first) ---
            dmy = wpool.tile([1, 256], bf16, name="dmy")
            nc.scalar.dma_start(out=dmy[:1, :], in_=wsb[:1, :256])
            b_mm = wpool.tile([128, 1024], bf16, name="b_mm")
            nc.scalar.dma_start(out=b_mm[:, :256], in_=wsb[:, :256])
            nc.scalar.dma_start(out=b_mm[:, 256:], in_=wsb[:, 256:])

            # --- x stream, FIFO on the SP ring in consumption order ---
            x0 = {}
            for bp in range(2):
                for h in range(2):
                    x0[(bp, h)] = xpool.tile([128, M_GROUP], f8e3,
                                             name=f"x0_{bp}{h}", tag="xg0")
            x1 = {}
            for bp in range(2):
                x1[bp] = xpool.tile([128, 2 * M_GROUP], f8e3,
                                    name=f"x1_{bp}", tag="xg1", bufs=2)

            nc.sync.dma_start(out=x0[(0, 0)][:, :HALF], in_=xpk0[0, 0][:, :HALF])
            nc.sync.dma_start(out=x0[(0, 0)][:, HALF:], in_=xpk0[0, 0][:, HALF:])
            nc.sync.dma_start(out=x0[(0, 1)][:, :HALF], in_=xpk0[0, 1][:, :HALF])
            nc.sync.dma_start(out=x0[(0, 1)][:, HALF:], in_=xpk0[0, 1][:, HALF:])
            nc.sync.dma_start(out=x0[(1, 0)][:], in_=xpk0[1, 0])
            nc.sync.dma_start(out=x0[(1, 1)][:], in_=xpk0[1, 1])

            # --- group-1 x on SWDGE, gated behind b2b's arrival so its
            # packets don't steal wire from the critical head. The gate is a
            # WAW dependency: a tiny DVE copy writes the first bytes of each
            # x1 tile (sourced from the 6th sync-ring DMA's data), so the
            # SWDGE dma_start -- which overwrites those bytes -- must wait. ---
            for bp in range(2):
                nc.vector.tensor_copy(x1[bp][:, :8],
                                      x0[(0, 1)][:, HALF:HALF + 8])
                nc.gpsimd.dma_start(out=x1[bp][:], in_=xpk1[bp])

            # --- PE warm-up: no data deps; starts the HAM activity window ---
            warm = wpool.tile([128, WARMUP_FREE], bf16, name="warm")
            nc.vector.memset(warm[:], 0)
            wps = pspool.tile([128, 2 * MM_FREE], f32, name="ps", tag="ps")
            for _ in range(WARMUP_MMS):
                nc.tensor.matmul(
                    wps[:, :WARMUP_FREE], lhsT=warm[:, :128], rhs=warm[:],
                    start=True, stop=True,
                )

            # --- matmuls: psum[n 128, m 1024] += b[k,n].T @ xT[k,m] ---
            # kc-outer over a pair of 2-bank psum tiles: one ldweights per 8
            # matmuls. Evacuations alternate DVE/ACT; stores alternate ACT/SP.
            ecnt = 0
            scnt = 0
            for g in range(N_GROUPS):
                for blk in range(BLOCKS_PER_CORE):
                    for ncol in range(2):  # n chunk of 128 within the block
                        out_sb = opool.tile([128, M_GROUP], i8, name="out_sb")
                        last = (g == N_GROUPS - 1 and blk == 1 and ncol == 1)
                        for mh2 in range(M_GROUP // (4 * MM_FREE)):
                            ps = [
                                pspool.tile([128, 2 * MM_FREE], f32, name="ps",
                                            tag="ps")
                                for _ in range(2)
                            ]
                            # zigzag kc across pairs: consecutive pairs end/
                            # start on the same stationary weight, halving
                            # ldweights switches (f32 a+b == b+a exactly)
                            kc_order = (0, 1) if mh2 % 2 == 0 else (1, 0)
                            for ki, kc in enumerate(kc_order):
                                lcol = ((blk * 2 + ncol) * 2 + kc) * 128
                                for t in range(2):
                                    for mi in range(2):
                                        mo = ((mh2 * 2 + t) * 2 + mi) * MM_FREE
                                        h = mo // HALF
                                        slot = kc if h == 0 else 1 - kc
                                        lo = slot * HALF + (mo % HALF)
                                        if g == 0:
                                            xt = x0[(blk, h)]
                                        else:
                                            xt = x1[blk]
                                            lo += h * M_GROUP
                                        nc.tensor.matmul(
                                            ps[t][:, mi * MM_FREE:(mi + 1) * MM_FREE],
                                            lhsT=b_mm[:, lcol:lcol + 128],
                                            rhs=xt[:, lo:lo + MM_FREE],
                                            start=(ki == 0),
                                            stop=(ki == 1),
                                        )
                            for t in range(2):
                                mo = (mh2 * 2 + t) * 2 * MM_FREE
                                dst = out_sb[:, mo:mo + 2 * MM_FREE]
                                if ecnt % 8 in DVE_EVAC_SLOTS:
                                    nc.vector.tensor_copy(dst, ps[t][:])
                                else:
                                    nc.scalar.copy(dst, ps[t][:])
                                ecnt += 1
                                if last:
                                    # final tile: store each quarter right
                                    # after its evacuation, alternating
                                    # rings, so the tail drain is 4 parallel
                                    # 128 KiB transfers instead of one
                                    # serial 512 KiB one
                                    eng = nc.scalar if t == 0 else nc.sync
                                    eng.dma_start(
                                        out=outTt[g, blk, ncol][
                                            :, mo:mo + 2 * MM_FREE],
                                        in_=out_sb[:, mo:mo + 2 * MM_FREE],
                                    )
                        if not last:
                            eng = nc.scalar if scnt % 2 == 0 else nc.sync
                            eng.dma_start(
                                out=outTt[g, blk, ncol], in_=out_sb[:],
                            )
                            scnt += 1
    nc.compile()
    return nc


def _get_nc():
    global _nc_cache
    if _nc_cache is None:
        _nc_cache = _build_nc()
    return _nc_cache


def _make_in_maps(x, blocks):
    # quantize x to fp8 e3m4 on the host (max |x| ~5.4 < 15.5, no clipping)
    xq = x.astype(ml_dtypes.float8_e3m4)
    xT = xq.T  # [4096, 8192] fp8 view
    x_std = float(x.std())
    in_maps = []
    s_o_all = np.empty(D, np.float32)
    for c in range(N_CORES):
        k0 = c * K_PER_CORE
        wsb = np.empty((128, 1024), np.float32)
        for blk in range(BLOCKS_PER_CORE):
            o = k0 + blk * BLOCK
            B = np.tanh(blocks[o:o + BLOCK, o:o + BLOCK])  # [256, 256]
            # per-column output scale: out[:,n] ~ N(0, x_std^2*||B[:,n]||^2)
            s_o = O_CLIP * np.sqrt((B * B).sum(0)) * x_std / 127.0
            s_o_all[o:o + BLOCK] = s_o
            Bs = B / s_o
            for ncol in range(2):
                for kc in range(2):
                    col = ((blk * 2 + ncol) * 2 + kc) * 128
                    wsb[:, col:col + 128] = \
                        Bs[kc * 128:(kc + 1) * 128,
                           ncol * 128:(ncol + 1) * 128]
        # consumption-ordered packed x
        shard = xT[k0:k0 + K_PER_CORE, :]              # [512, 8192]
        s4 = shard.reshape(4, 128, N_GROUPS, 2, HALF)  # [q, p, g, h, 2048]
        xpk0 = np.empty((2, 2, 128, M_GROUP), xq.dtype)
        xpk1 = np.empty((2, 128, 2 * M_GROUP), xq.dtype)
        for bp in range(2):
            for h in range(2):
                first = 2 * bp + (h & 1)
                second = 2 * bp + 1 - (h & 1)
                xpk0[bp, h, :, 0:HALF] = s4[first, :, 0, h]
                xpk0[bp, h, :, HALF:] = s4[second, :, 0, h]
                xpk1[bp, :, h * M_GROUP:h * M_GROUP + HALF] = s4[first, :, 1, h]
                xpk1[bp, :, h * M_GROUP + HALF:(h + 1) * M_GROUP] = \
                    s4[second, :, 1, h]
        in_maps.append({
            "xpk0": xpk0,
            "xpk1": xpk1,
            "wsb": wsb.astype(ml_dtypes.bfloat16),
        })
    return in_maps, s_o_all


def _run(x, blocks, **spmd_kwargs):
    in_maps, s_o = _make_in_maps(x, blocks)
    res = run_bass_kernel_spmd(
        _get_nc(), in_maps, core_ids=list(range(N_CORES)),
        **spmd_kwargs,
    )
    out = np.empty((N_ROWS, D), np.float32)
    for c in range(N_CORES):
        cols = slice(c * K_PER_CORE, (c + 1) * K_PER_CORE)
        # outTt [g, blk, ncol, 128, M_GROUP] -> outT [512, 8192]
        ot = res.results[c]["outTt"]
        shard = ot.transpose(1, 2, 3, 0, 4).reshape(K_PER_CORE, N_ROWS)
        shard = shard.T.astype(np.float32)
        out[:, cols] = shard * s_o[cols]
    return out, res


def kernel(x, blocks, mask=None):
    out, _ = _run(np.asarray(x), np.asarray(blocks))
    return out
